# revision 6
# baseline (speedup 1.0000x reference)
"""Cosformer attention (causal linear attention with cos reweighting) on 8
Trainium2 NeuronCores.

Sharding: n = bsz*heads = 16 sequences -> 2 per core. Core c handles batch-half
i = c//4 and head-pair p = c%4 (heads 2p, 2p+1). Fully data/head parallel; the
only cross-core interaction is the host-side sum of output-projection partials.

Per-core kernel (L=1024 tokens, d=64 per head, pair feature dim P=128):
  1. Feat-major projections Q^T/K^T per head (duplicated-W trick: weight slice
     [Wh.T | Wh.T] (512x128) so that rows 0:64 and 64:128 both hold the head's
     features) -> relu(+bias) -> elementwise * [sin;cos] row table = q_^T, k_^T.
     V^T projected once per pair (128 feats).
  2. Chunked causal linear attention, chunk=128: per chunk
       B   = (k_^T chunk).T-style matmul -> A^T, masked upper-tri  (j<=i)
       qkv = B.T @ V~  +  q_chunk.T-style inter with running state S (128x65)
       V~ = [V | 1] so column 64 accumulates the denominator.
       state S += K_tok^T @ V~ (K token-major via PE transpose).
       attn = qkv[:,0:64] / max(denom,eps)  (per-partition scalars).
  3. attn pair chunk (128x128) -> PE transpose -> out-proj partial
     (128x512) = attn^T.T @ WoT-slice -> DMA to DRAM.
Host sums 4 partials per batch-half, adds bo, reinterleaves rows.
"""

import os
import sys

import numpy as np

for _p in ("/opt/trn_rl_repo", "/root/.axon_site/_ro/trn_rl_repo"):
    if os.path.isdir(_p) and _p not in sys.path:
        sys.path.insert(0, _p)

N_HEAD = 8
E = 512
L = 1024  # sequence length per batch-half
BSZ = 2
D = 64  # head dim
P = 128  # partition/chunk/pair-feature size
NCHUNK = L // P
EPS = 1e-6
N_CORES = 8
TH = 512  # token-half width for projections (fp32 moving max)

_CACHE = {}


def _build_bass():
    import concourse.bass as bass
    import concourse.tile as tile
    from concourse import bacc, mybir
    from contextlib import ExitStack

    f32 = mybir.dt.float32
    f32r = mybir.dt.float32r
    AF = mybir.ActivationFunctionType

    nc = bacc.Bacc("TRN2", target_bir_lowering=False, debug=False)

    xt_d = nc.dram_tensor("xt", [E, L], f32, kind="ExternalInput")
    w_d = {}
    for nm in ("wq_a", "wq_b", "wk_a", "wk_b", "wv"):
        w_d[nm] = nc.dram_tensor(nm, [E, P], f32, kind="ExternalInput")
    wo_d = nc.dram_tensor("wo", [P, E], f32, kind="ExternalInput")
    b_d = {}
    for nm in ("bq_a", "bq_b", "bk_a", "bk_b", "bv"):
        b_d[nm] = nc.dram_tensor(nm, [P, 1], f32, kind="ExternalInput")
    scb_d = nc.dram_tensor("scb", [P, L], f32, kind="ExternalInput")
    mask_d = nc.dram_tensor("mask", [P, P], f32, kind="ExternalInput")
    ident_d = nc.dram_tensor("ident", [P, P], f32, kind="ExternalInput")
    out_d = nc.dram_tensor("out", [L, E], f32, kind="ExternalOutput")

    def R(ap):
        # f32r (TF32) requires producers to emit f32r-typed outputs; plain
        # f32 for now — selectively re-enabled via dtype declarations.
        return ap

    with tile.TileContext(nc) as tc:
        with ExitStack() as ctx:
            ep = ctx.enter_context
            cpool = ep(tc.tile_pool(name="const", bufs=1))
            seqp = ep(tc.tile_pool(name="seq", bufs=1))
            ktokp = ep(tc.tile_pool(name="ktok", bufs=4))
            vtp = ep(tc.tile_pool(name="vt", bufs=4))
            bp = ep(tc.tile_pool(name="bsb", bufs=3))
            sp = ep(tc.tile_pool(name="state", bufs=4))
            app = ep(tc.tile_pool(name="apair", bufs=2))
            atp = ep(tc.tile_pool(name="attnT", bufs=2))
            outp = ep(tc.tile_pool(name="outsb", bufs=2))
            rp = ep(tc.tile_pool(name="rcol", bufs=4))
            big_ps = ep(tc.tile_pool(name="bigps", bufs=2, space="PSUM"))
            sq_ps = ep(tc.tile_pool(name="sqps", bufs=3, space="PSUM"))
            acc_ps = ep(tc.tile_pool(name="accps", bufs=3, space="PSUM"))

            # ---- constant loads ----
            xts = []
            for e in range(4):
                t = cpool.tile([P, L], f32, name=f"xt{e}")
                nc.sync.dma_start(t[:], xt_d[e * P : (e + 1) * P, :])
                xts.append(t)
            wt = {}
            for nm in ("wq_a", "wq_b", "wk_a", "wk_b", "wv"):
                wt[nm] = []
                for e in range(4):
                    t = cpool.tile([P, P], f32, name=f"{nm}{e}")
                    nc.sync.dma_start(t[:], w_d[nm][e * P : (e + 1) * P, :])
                    wt[nm].append(t)
            wo_t = cpool.tile([P, E], f32, name="wo_t")
            nc.sync.dma_start(wo_t[:], wo_d[:, :])
            bt = {}
            for nm in ("bq_a", "bq_b", "bk_a", "bk_b", "bv"):
                bt[nm] = cpool.tile([P, 1], f32, name=f"{nm}_t")
                nc.sync.dma_start(bt[nm][:], b_d[nm][:, :])
            scb_t = cpool.tile([P, L], f32, name="scb_t")
            nc.sync.dma_start(scb_t[:], scb_d[:, :])
            mask_t = cpool.tile([P, P], f32, name="mask_t")
            nc.sync.dma_start(mask_t[:], mask_d[:, :])
            ident_t = cpool.tile([P, P], f32, name="ident_t")
            nc.sync.dma_start(ident_t[:], ident_d[:, :])

            # ---- projections (feat-major) ----
            def project(wname, bname, func, outname, do_scale):
                seq = seqp.tile([P, L], f32, name=outname)
                for th in range(L // TH):
                    ps = big_ps.tile(
                        [P, TH], f32, tag="big", name=f"{outname}_ps{th}"
                    )
                    for e in range(4):
                        nc.tensor.matmul(
                            ps[:],
                            R(wt[wname][e][:]),
                            R(xts[e][:, th * TH : (th + 1) * TH]),
                            start=(e == 0),
                            stop=(e == 3),
                        )
                    sl = seq[:, th * TH : (th + 1) * TH]
                    nc.scalar.activation(sl, ps[:], func, bias=bt[bname][:, 0:1])
                    if do_scale:
                        nc.vector.tensor_mul(
                            sl, sl, scb_t[:, th * TH : (th + 1) * TH]
                        )
                return seq

            q_seq = {}
            k_seq = {}
            for j, h in enumerate("ab"):
                q_seq[h] = project(f"wq_{h}", f"bq_{h}", AF.Relu, f"q_{h}", True)
                k_seq[h] = project(f"wk_{h}", f"bk_{h}", AF.Relu, f"k_{h}", True)
            v_seq = project("wv", "bv", AF.Identity, "v_pair", False)

            # ---- attention ----
            S_prev = {"a": None, "b": None}
            for c in range(NCHUNK):
                cs = slice(c * P, (c + 1) * P)
                # V token-major for this chunk (both heads) + ones column
                vt_ps = sq_ps.tile([P, P], f32, tag="sq", name=f"vtps{c}")
                nc.tensor.matmul(
                    vt_ps[:], v_seq[:, cs], ident_t[:], is_transpose=True
                )
                vtile = {}
                for j, h in enumerate("ab"):
                    vt = vtp.tile([P, D + 1], f32, tag="vt", name=f"vt_{h}{c}")
                    nc.scalar.copy(vt[:, 0:D], vt_ps[:, j * D : (j + 1) * D])
                    nc.vector.memset(vt[:, D : D + 1], 1.0)
                    vtile[h] = vt

                attn_pair = app.tile([P, P], f32, tag="ap", name=f"ap{c}")
                for j, h in enumerate("ab"):
                    qc = q_seq[h][:, cs]
                    kc = k_seq[h][:, cs]
                    # masked A^T
                    b_ps = sq_ps.tile([P, P], f32, tag="sq", name=f"bps_{h}{c}")
                    nc.tensor.matmul(b_ps[:], R(kc), R(qc), start=True, stop=True)
                    b_sb = bp.tile([P, P], f32, tag="bsb", name=f"bsb_{h}{c}")
                    nc.vector.tensor_mul(b_sb[:], b_ps[:], mask_t[:])
                    # qkv = intra + inter
                    qkv = acc_ps.tile([P, D + 1], f32, tag="acc", name=f"qkv_{h}{c}")
                    nc.tensor.matmul(
                        qkv[:], R(b_sb[:]), R(vtile[h][:]),
                        start=True, stop=(c == 0),
                    )
                    if c > 0:
                        nc.tensor.matmul(
                            qkv[:], R(qc), R(S_prev[h][:]),
                            start=False, stop=True,
                        )
                    # state update (skip on last chunk)
                    if c < NCHUNK - 1:
                        kt_ps = sq_ps.tile([P, P], f32, tag="sq", name=f"ktps_{h}{c}")
                        nc.tensor.matmul(
                            kt_ps[:], kc, ident_t[:], is_transpose=True
                        )
                        ktok = ktokp.tile([P, P], f32, tag="ktok", name=f"ktok_{h}{c}")
                        nc.scalar.copy(ktok[:], kt_ps[:])
                        d_ps = acc_ps.tile([P, D + 1], f32, tag="acc", name=f"dps_{h}{c}")
                        nc.tensor.matmul(
                            d_ps[:], R(ktok[:]), R(vtile[h][:]),
                            start=True, stop=True,
                        )
                        s_new = sp.tile([P, D + 1], f32, tag="S", name=f"S_{h}{c}")
                        if c == 0:
                            nc.scalar.copy(s_new[:], d_ps[:])
                        else:
                            nc.vector.tensor_add(s_new[:], S_prev[h][:], d_ps[:])
                        S_prev[h] = s_new
                    # normalize
                    r_col = rp.tile([P, 2], f32, tag="r", name=f"r_{h}{c}")
                    nc.vector.tensor_scalar_max(r_col[:, 0:1], qkv[:, D : D + 1], EPS)
                    nc.vector.reciprocal(r_col[:, 1:2], r_col[:, 0:1])
                    nc.vector.tensor_scalar_mul(
                        attn_pair[:, j * D : (j + 1) * D], qkv[:, 0:D], r_col[:, 1:2]
                    )
                # out projection for this chunk
                at_ps = sq_ps.tile([P, P], f32, tag="sq", name=f"atps{c}")
                nc.tensor.matmul(
                    at_ps[:], attn_pair[:], ident_t[:], is_transpose=True
                )
                at_sb = atp.tile([P, P], f32, tag="at", name=f"at{c}")
                nc.scalar.copy(at_sb[:], at_ps[:])
                o_ps = big_ps.tile([P, E], f32, tag="big", name=f"ops{c}")
                nc.tensor.matmul(
                    o_ps[:], R(at_sb[:]), R(wo_t[:]), start=True, stop=True
                )
                o_sb = outp.tile([P, E], f32, tag="osb", name=f"osb{c}")
                nc.scalar.copy(o_sb[:], o_ps[:])
                nc.sync.dma_start(out_d[cs, :], o_sb[:])

    nc.compile()
    return nc


def _get_nc():
    if "nc" not in _CACHE:
        _CACHE["nc"] = _build_bass()
    return _CACHE["nc"]


def make_in_maps(query, Wq, bq, Wk, bk, Wv, bv, Wo, bo):
    f32 = np.float32
    query = np.asarray(query, f32)
    x3 = query.reshape(L, BSZ, E)  # faithful torch .view reshape
    idx = (np.pi / 2) * np.arange(1, L + 1, dtype=f32) / f32(L)
    sinv = np.sin(idx).astype(f32)
    cosv = np.cos(idx).astype(f32)
    scb = np.empty((P, L), f32)
    scb[0:D] = sinv[None, :]
    scb[D:P] = cosv[None, :]
    mask = np.triu(np.ones((P, P), f32))
    ident = np.eye(P, dtype=f32)

    Wq, Wk, Wv, Wo = (np.asarray(w, f32) for w in (Wq, Wk, Wv, Wo))
    bq, bk, bv = (np.asarray(b, f32) for b in (bq, bk, bv))

    def wslice(W, h):
        w = W[D * h : D * (h + 1), :].T  # (512, 64)
        return np.ascontiguousarray(np.concatenate([w, w], axis=1))

    def bslice(b, h):
        bb = b[D * h : D * (h + 1)]
        return np.concatenate([bb, bb]).reshape(P, 1).astype(f32)

    in_maps = []
    for c in range(N_CORES):
        i, p = divmod(c, 4)
        hA, hB = 2 * p, 2 * p + 1
        in_maps.append(
            dict(
                xt=np.ascontiguousarray(x3[:, i, :].T),
                wq_a=wslice(Wq, hA), wq_b=wslice(Wq, hB),
                wk_a=wslice(Wk, hA), wk_b=wslice(Wk, hB),
                wv=np.ascontiguousarray(Wv[P * p : P * (p + 1), :].T),
                bq_a=bslice(bq, hA), bq_b=bslice(bq, hB),
                bk_a=bslice(bk, hA), bk_b=bslice(bk, hB),
                bv=bv[P * p : P * (p + 1)].reshape(P, 1).astype(f32),
                wo=np.ascontiguousarray(Wo[:, P * p : P * (p + 1)].T),
                scb=scb, mask=mask, ident=ident,
            )
        )
    return in_maps


def assemble(partials, bo):
    out_flat = np.zeros((BSZ * L, E), np.float32)
    out_flat[0::2] = partials[0] + partials[1] + partials[2] + partials[3]
    out_flat[1::2] = partials[4] + partials[5] + partials[6] + partials[7]
    out_flat += np.asarray(bo, np.float32)[None, :]
    return out_flat.reshape(BSZ, L, E)


def run(inputs, trace=False):
    from concourse.bass_utils import run_bass_kernel_spmd

    in_maps = make_in_maps(**inputs)
    nc = _get_nc()
    res = run_bass_kernel_spmd(nc, in_maps, list(range(N_CORES)), trace=trace)
    partials = [r["out"] for r in res.results]
    return assemble(partials, inputs["bo"]), res


def kernel(**inputs):
    out, _ = run(inputs, trace=False)
    return out


# revision 16
# speedup vs baseline: 1.2892x; 1.2892x over previous
"""Cosformer attention (causal linear attention with cos reweighting) on 8
Trainium2 NeuronCores.

Sharding: n = bsz*heads = 16 sequences -> 2 per core. Core c handles batch-half
i = c//4 and head-pair p = c%4 (heads 2p, 2p+1). Fully data/head parallel; the
only cross-core interaction is the host-side sum of output-projection partials.

Per-core kernel (L=1024 tokens, d=64 per head, pair feature dim P=128):
  1. Feat-major projections Q^T/K^T per head (duplicated-W trick: weight slice
     [Wh.T | Wh.T] (512x128) so that rows 0:64 and 64:128 both hold the head's
     features) -> relu(+bias) -> elementwise * [sin;cos] row table = q_^T, k_^T.
     V^T projected once per pair (128 feats).
  2. Chunked causal linear attention, chunk=128: per chunk
       B   = (k_^T chunk).T-style matmul -> A^T, masked upper-tri  (j<=i)
       qkv = B.T @ V~  +  q_chunk.T-style inter with running state S (128x65)
       V~ = [V | 1] so column 64 accumulates the denominator.
       state S += K_tok^T @ V~ (K token-major via PE transpose).
       attn = qkv[:,0:64] / max(denom,eps)  (per-partition scalars).
  3. attn pair chunk (128x128) -> PE transpose -> out-proj partial
     (128x512) = attn^T.T @ WoT-slice -> DMA to DRAM.
Host sums 4 partials per batch-half, adds bo, reinterleaves rows.
"""

import os
import sys

import numpy as np

for _p in ("/opt/trn_rl_repo", "/root/.axon_site/_ro/trn_rl_repo"):
    if os.path.isdir(_p) and _p not in sys.path:
        sys.path.insert(0, _p)

N_HEAD = 8
E = 512
L = 1024  # sequence length per batch-half
BSZ = 2
D = 64  # head dim
P = 128  # partition/chunk/pair-feature size
NCHUNK = L // P
EPS = 1e-6
N_CORES = 8
TH = 512  # token-half width for projections (fp32 moving max)

_CACHE = {}


def _build_bass():
    import concourse.bass as bass
    import concourse.tile as tile
    from concourse import bacc, mybir
    from contextlib import ExitStack

    f32 = mybir.dt.float32
    f32r = mybir.dt.float32r
    bf16 = mybir.dt.bfloat16
    AF = mybir.ActivationFunctionType

    nc = bacc.Bacc("TRN2", target_bir_lowering=False, debug=False)

    xt_d = nc.dram_tensor("xt", [E, L], f32r, kind="ExternalInput")
    w_d = {}
    for nm in ("wq_a", "wq_b", "wk_a", "wk_b", "wv"):
        w_d[nm] = nc.dram_tensor(nm, [E, P], f32r, kind="ExternalInput")
    wo_d = nc.dram_tensor("wo", [P, E], f32r, kind="ExternalInput")
    b_d = {}
    for nm in ("bq_a", "bq_b", "bk_a", "bk_b", "bv"):
        b_d[nm] = nc.dram_tensor(nm, [P, 1], f32, kind="ExternalInput")
    scb_d = nc.dram_tensor("scb", [P, L], f32, kind="ExternalInput")
    mask_d = nc.dram_tensor("mask", [P, P], f32, kind="ExternalInput")
    ident_d = nc.dram_tensor("ident", [P, P], bf16, kind="ExternalInput")
    out_d = nc.dram_tensor("out", [L, E], f32, kind="ExternalOutput")

    def R(ap):
        # f32r (TF32) requires producers to emit f32r-typed outputs; plain
        # f32 for now — selectively re-enabled via dtype declarations.
        return ap

    with tile.TileContext(nc) as tc:
        with ExitStack() as ctx:
            ep = ctx.enter_context
            cpool = ep(tc.tile_pool(name="const", bufs=1))
            seqp = ep(tc.tile_pool(name="seq", bufs=1))
            ktokp = ep(tc.tile_pool(name="ktok", bufs=4))
            vtp = ep(tc.tile_pool(name="vt", bufs=4))
            bp = ep(tc.tile_pool(name="bsb", bufs=3))
            sp = ep(tc.tile_pool(name="state", bufs=4))
            app = ep(tc.tile_pool(name="apair", bufs=2))
            atp = ep(tc.tile_pool(name="attnT", bufs=2))
            outp = ep(tc.tile_pool(name="outsb", bufs=2))
            rp = ep(tc.tile_pool(name="rcol", bufs=4))
            big_ps = ep(tc.tile_pool(name="bigps", bufs=2, space="PSUM"))
            sq_ps = ep(tc.tile_pool(name="sqps", bufs=2, space="PSUM"))
            acc_ps = ep(tc.tile_pool(name="accps", bufs=2, space="PSUM"))
            s_ps = ep(tc.tile_pool(name="sps", bufs=1, space="PSUM"))

            # ---- constant loads ----
            xts = []
            for e in range(4):
                t = cpool.tile([P, L], f32r, name=f"xt{e}")
                for th in range(L // TH):
                    nc.sync.dma_start(
                        t[:, th * TH : (th + 1) * TH],
                        xt_d[e * P : (e + 1) * P, th * TH : (th + 1) * TH],
                    )
                xts.append(t)
            wt = {}
            for nm in ("wq_a", "wq_b", "wk_a", "wk_b", "wv"):
                wt[nm] = []
                for e in range(4):
                    t = cpool.tile([P, P], f32r, name=f"{nm}{e}")
                    nc.sync.dma_start(t[:], w_d[nm][e * P : (e + 1) * P, :])
                    wt[nm].append(t)
            wo_t = cpool.tile([P, E], f32r, name="wo_t")
            nc.sync.dma_start(wo_t[:], wo_d[:, :])
            bt = {}
            for nm in ("bq_a", "bq_b", "bk_a", "bk_b", "bv"):
                bt[nm] = cpool.tile([P, 1], f32, name=f"{nm}_t")
                nc.sync.dma_start(bt[nm][:], b_d[nm][:, :])
            scb_t = cpool.tile([P, L], f32, name="scb_t")
            nc.sync.dma_start(scb_t[:], scb_d[:, :])
            mask_t = cpool.tile([P, P], f32, name="mask_t")
            nc.sync.dma_start(mask_t[:], mask_d[:, :])
            ident_t = cpool.tile([P, P], bf16, name="ident_t")
            nc.sync.dma_start(ident_t[:], ident_d[:, :])

            # ---- projections (feat-major) ----
            def project(wname, bname, func, outname, do_scale):
                seq = seqp.tile([P, L], bf16, name=outname)
                for th in range(L // TH):
                    ps = big_ps.tile(
                        [P, TH], f32, tag="big", name=f"{outname}_ps{th}"
                    )
                    for e in range(4):
                        nc.tensor.matmul(
                            ps[:],
                            R(wt[wname][e][:]),
                            R(xts[e][:, th * TH : (th + 1) * TH]),
                            start=(e == 0),
                            stop=(e == 3),
                        )
                    sl = seq[:, th * TH : (th + 1) * TH]
                    nc.scalar.activation(sl, ps[:], func, bias=bt[bname][:, 0:1])
                    if do_scale:
                        nc.vector.tensor_mul(
                            sl, sl, scb_t[:, th * TH : (th + 1) * TH]
                        )
                return seq

            q_seq = {}
            k_seq = {}
            for j, h in enumerate("ab"):
                q_seq[h] = project(f"wq_{h}", f"bq_{h}", AF.Relu, f"q_{h}", True)
                k_seq[h] = project(f"wk_{h}", f"bk_{h}", AF.Relu, f"k_{h}", True)
            v_seq = project("wv", "bv", AF.Identity, "v_pair", False)

            # ---- attention ----
            # Per-head fp32 state accumulated in a dedicated PSUM bank via an
            # interleaved matmul accumulation group; bf16 snapshots feed the
            # inter matmul of the next chunk.
            S_psum = {}
            for h in "ab":
                S_psum[h] = s_ps.tile([P, D + 1], f32, name=f"Sps_{h}")
            S_prev = {"a": None, "b": None}
            for c in range(NCHUNK):
                cs = slice(c * P, (c + 1) * P)
                # V token-major for this chunk (both heads) + ones column
                vt_ps = sq_ps.tile([P, P], bf16, tag="sq", name=f"vtps{c}")
                nc.tensor.matmul(
                    vt_ps[:], v_seq[:, cs], ident_t[:], is_transpose=True
                )
                vtile = {}
                for j, h in enumerate("ab"):
                    vt = vtp.tile([P, D + 1], bf16, tag="vt", name=f"vt_{h}{c}")
                    nc.scalar.copy(vt[:, 0:D], vt_ps[:, j * D : (j + 1) * D])
                    nc.vector.memset(vt[:, D : D + 1], 1.0)
                    vtile[h] = vt

                attn_pair = app.tile([P, P], bf16, tag="ap", name=f"ap{c}")
                for j, h in enumerate("ab"):
                    qc = q_seq[h][:, cs]
                    kc = k_seq[h][:, cs]
                    # masked A^T
                    b_ps = sq_ps.tile([P, P], f32, tag="sq", name=f"bps_{h}{c}")
                    nc.tensor.matmul(b_ps[:], kc, qc, start=True, stop=True)
                    b_sb = bp.tile([P, P], bf16, tag="bsb", name=f"bsb_{h}{c}")
                    nc.vector.tensor_mul(b_sb[:], b_ps[:], mask_t[:])
                    # qkv = intra + inter
                    qkv = acc_ps.tile([P, D + 1], f32, tag="acc", name=f"qkv_{h}{c}")
                    nc.tensor.matmul(
                        qkv[:], b_sb[:], vtile[h][:],
                        start=True, stop=(c == 0),
                    )
                    if c > 0:
                        nc.tensor.matmul(
                            qkv[:], qc, S_prev[h][:],
                            start=False, stop=True,
                        )
                    # state update (skip on last chunk): accumulate in PSUM
                    if c < NCHUNK - 1:
                        kt_ps = sq_ps.tile([P, P], bf16, tag="sq", name=f"ktps_{h}{c}")
                        nc.tensor.matmul(
                            kt_ps[:], kc, ident_t[:], is_transpose=True
                        )
                        ktok = ktokp.tile([P, P], bf16, tag="ktok", name=f"ktok_{h}{c}")
                        nc.scalar.copy(ktok[:], kt_ps[:])
                        nc.tensor.matmul(
                            S_psum[h][:], ktok[:], vtile[h][:],
                            start=(c == 0), stop=(c == NCHUNK - 2),
                            skip_group_check=True,
                        )
                        s_new = sp.tile([P, D + 1], bf16, tag="S", name=f"S_{h}{c}")
                        nc.scalar.copy(s_new[:], S_psum[h][:])
                        S_prev[h] = s_new
                    # normalize
                    r_col = rp.tile([P, 2], f32, tag="r", name=f"r_{h}{c}")
                    nc.vector.tensor_scalar_max(r_col[:, 0:1], qkv[:, D : D + 1], EPS)
                    nc.vector.reciprocal(r_col[:, 1:2], r_col[:, 0:1])
                    nc.vector.tensor_scalar_mul(
                        attn_pair[:, j * D : (j + 1) * D], qkv[:, 0:D], r_col[:, 1:2]
                    )
                # out projection for this chunk
                at_ps = sq_ps.tile([P, P], bf16, tag="sq", name=f"atps{c}")
                nc.tensor.matmul(
                    at_ps[:], attn_pair[:], ident_t[:], is_transpose=True
                )
                at_sb = atp.tile([P, P], f32r, tag="at", name=f"at{c}")
                nc.scalar.copy(at_sb[:], at_ps[:])
                o_ps = big_ps.tile([P, E], f32, tag="big", name=f"ops{c}")
                nc.tensor.matmul(
                    o_ps[:], R(at_sb[:]), R(wo_t[:]), start=True, stop=True
                )
                o_sb = outp.tile([P, E], f32, tag="osb", name=f"osb{c}")
                nc.scalar.copy(o_sb[:], o_ps[:])
                nc.sync.dma_start(out_d[cs, :], o_sb[:])

    nc.compile()
    return nc


def _get_nc():
    if "nc" not in _CACHE:
        _CACHE["nc"] = _build_bass()
    return _CACHE["nc"]


def make_in_maps(query, Wq, bq, Wk, bk, Wv, bv, Wo, bo):
    f32 = np.float32
    query = np.asarray(query, f32)
    x3 = query.reshape(L, BSZ, E)  # faithful torch .view reshape
    idx = (np.pi / 2) * np.arange(1, L + 1, dtype=f32) / f32(L)
    sinv = np.sin(idx).astype(f32)
    cosv = np.cos(idx).astype(f32)
    scb = np.empty((P, L), f32)
    scb[0:D] = sinv[None, :]
    scb[D:P] = cosv[None, :]
    import ml_dtypes

    mask = np.triu(np.ones((P, P), f32))
    ident = np.eye(P, dtype=ml_dtypes.bfloat16)

    Wq, Wk, Wv, Wo = (np.asarray(w, f32) for w in (Wq, Wk, Wv, Wo))
    bq, bk, bv = (np.asarray(b, f32) for b in (bq, bk, bv))

    def wslice(W, h):
        w = W[D * h : D * (h + 1), :].T  # (512, 64)
        return np.ascontiguousarray(np.concatenate([w, w], axis=1))

    def bslice(b, h):
        bb = b[D * h : D * (h + 1)]
        return np.concatenate([bb, bb]).reshape(P, 1).astype(f32)

    in_maps = []
    for c in range(N_CORES):
        i, p = divmod(c, 4)
        hA, hB = 2 * p, 2 * p + 1
        in_maps.append(
            dict(
                xt=np.ascontiguousarray(x3[:, i, :].T),
                wq_a=wslice(Wq, hA), wq_b=wslice(Wq, hB),
                wk_a=wslice(Wk, hA), wk_b=wslice(Wk, hB),
                wv=np.ascontiguousarray(Wv[P * p : P * (p + 1), :].T),
                bq_a=bslice(bq, hA), bq_b=bslice(bq, hB),
                bk_a=bslice(bk, hA), bk_b=bslice(bk, hB),
                bv=bv[P * p : P * (p + 1)].reshape(P, 1).astype(f32),
                wo=np.ascontiguousarray(Wo[:, P * p : P * (p + 1)].T),
                scb=scb, mask=mask, ident=ident,
            )
        )
    return in_maps


def assemble(partials, bo):
    out_flat = np.zeros((BSZ * L, E), np.float32)
    out_flat[0::2] = partials[0] + partials[1] + partials[2] + partials[3]
    out_flat[1::2] = partials[4] + partials[5] + partials[6] + partials[7]
    out_flat += np.asarray(bo, np.float32)[None, :]
    return out_flat.reshape(BSZ, L, E)


def run(inputs, trace=False):
    from concourse.bass_utils import run_bass_kernel_spmd

    in_maps = make_in_maps(**inputs)
    nc = _get_nc()
    res = run_bass_kernel_spmd(nc, in_maps, list(range(N_CORES)), trace=trace)
    partials = [r["out"] for r in res.results]
    return assemble(partials, inputs["bo"]), res


def kernel(**inputs):
    out, _ = run(inputs, trace=False)
    return out


# revision 21
# speedup vs baseline: 1.5144x; 1.1747x over previous
"""Cosformer attention (causal linear attention with cos reweighting) on 8
Trainium2 NeuronCores.

Sharding: n = bsz*heads = 16 sequences -> 2 per core. Core c handles batch-half
i = c//4 and head-pair p = c%4 (heads 2p, 2p+1). Fully data/head parallel; the
only cross-core interaction is the host-side sum of output-projection partials.

Per-core kernel (L=1024 tokens, d=64 per head, pair feature dim P=128):
  1. Feat-major projections Q^T/K^T per head in TF32 (duplicated-W trick:
     weight slice [Wh.T | Wh.T] (512x128) so rows 0:64 / 64:128 both hold the
     head's features) -> relu(+bias) -> * [sin;cos] row table -> bf16 q_^T,k_^T.
     V^T projected once per pair.
  2. Chunked causal linear attention (bf16 matmuls, fp32 PSUM), chunk=128:
       B   = masked A^T (upper-tri j<=i)
       qkv = B.T @ V~  +  q^T.T @ S      (V~ = [V|1]; col 64 = denominator)
       S  += K_tok.T @ V~ in a persistent PSUM bank (fp32, no drift)
       attn = qkv[:,0:64] * 1/max(denom,eps)   (per-partition scalars)
  3. attn pair chunk -> PE transpose -> TF32 out-proj partial -> DRAM.
Host sums 4 partials per batch-half, adds bo, reinterleaves rows.

Inputs per core are packed into three DMA images (xt + 2 constant packs) to
minimize DMA trigger count; loads are split across the two HWDGE rings
(sync/SP and scalar/ACT); output partials go out via gpsimd SWDGE.
"""

import os
import sys

import numpy as np

for _p in ("/opt/trn_rl_repo", "/root/.axon_site/_ro/trn_rl_repo"):
    if os.path.isdir(_p) and _p not in sys.path:
        sys.path.insert(0, _p)

N_HEAD = 8
E = 512
L = 1024  # sequence length per batch-half
BSZ = 2
D = 64  # head dim
P = 128  # partition/chunk/pair-feature size
NCHUNK = L // P
EPS = 1e-6
N_CORES = 8
TH = 512  # token-half width for projections (f32r moving max)

# rest-pack column offsets (fp32 columns)
_WV_OFF = 0
_WO_OFF = 512
_SCB_OFF = 1024
_MASK_OFF = 2048
_BIAS_OFF = 2176  # 5 cols: bq_a, bq_b, bk_a, bk_b, bv
_IDENT_OFF = 2181  # 64 f32 cols = 128 bf16 cols
_REST_COLS = 2245

_CACHE = {}


def _build_bass():
    import concourse.bass as bass
    import concourse.tile as tile
    from concourse import bacc, mybir
    from contextlib import ExitStack

    f32 = mybir.dt.float32
    f32r = mybir.dt.float32r
    bf16 = mybir.dt.bfloat16
    AF = mybir.ActivationFunctionType

    nc = bacc.Bacc("TRN2", target_bir_lowering=False, debug=False)

    xt_d = nc.dram_tensor("xt", [E, L], f32r, kind="ExternalInput")
    cw_d = nc.dram_tensor("cw", [P, 2048], f32r, kind="ExternalInput")
    rp_d = nc.dram_tensor("rp", [P, _REST_COLS], f32r, kind="ExternalInput")
    out_d = nc.dram_tensor("out", [L, E], f32, kind="ExternalOutput")

    with tile.TileContext(nc) as tc:
        with ExitStack() as ctx:
            ep = ctx.enter_context
            cpool = ep(tc.tile_pool(name="const", bufs=1))
            seqp = ep(tc.tile_pool(name="seq", bufs=1))
            ktokp = ep(tc.tile_pool(name="ktok", bufs=4))
            vtp = ep(tc.tile_pool(name="vt", bufs=4))
            bp = ep(tc.tile_pool(name="bsb", bufs=3))
            sp = ep(tc.tile_pool(name="state", bufs=4))
            app = ep(tc.tile_pool(name="apair", bufs=2))
            atp = ep(tc.tile_pool(name="attnT", bufs=2))
            outp = ep(tc.tile_pool(name="outsb", bufs=2))
            rp = ep(tc.tile_pool(name="rcol", bufs=4))
            big_ps = ep(tc.tile_pool(name="bigps", bufs=2, space="PSUM"))
            sq_ps = ep(tc.tile_pool(name="sqps", bufs=2, space="PSUM"))
            acc_ps = ep(tc.tile_pool(name="accps", bufs=2, space="PSUM"))
            s_ps = ep(tc.tile_pool(name="sps", bufs=1, space="PSUM"))

            # ---- loads: two HWDGE rings (sync + scalar) ----
            cw_t = cpool.tile([P, 2048], f32r, name="cw_t")
            nc.sync.dma_start(cw_t[:], cw_d[:, :])
            rp_t = cpool.tile([P, _REST_COLS], f32r, name="rp_t")
            nc.scalar.dma_start(rp_t[:], rp_d[:, :])
            xts = []
            for e in range(4):
                t = cpool.tile([P, L], f32r, name=f"xt{e}")
                nc.sync.dma_start(t[:, 0:TH], xt_d[e * P : (e + 1) * P, 0:TH])
                nc.scalar.dma_start(t[:, TH:L], xt_d[e * P : (e + 1) * P, TH:L])
                xts.append(t)

            wt = {}
            for wi, nm in enumerate(("wq_a", "wq_b", "wk_a", "wk_b")):
                wt[nm] = [
                    cw_t[:, wi * 512 + e * P : wi * 512 + (e + 1) * P]
                    for e in range(4)
                ]
            wt["wv"] = [
                rp_t[:, _WV_OFF + e * P : _WV_OFF + (e + 1) * P] for e in range(4)
            ]
            wo_t = rp_t[:, _WO_OFF : _WO_OFF + 512]
            scb_t = rp_t[:, _SCB_OFF : _SCB_OFF + 1024].bitcast(f32)
            mask_t = rp_t[:, _MASK_OFF : _MASK_OFF + 128].bitcast(f32)
            bnames = ("bq_a", "bq_b", "bk_a", "bk_b", "bv")
            bt = {
                nm: rp_t[:, _BIAS_OFF + i : _BIAS_OFF + i + 1].bitcast(f32)
                for i, nm in enumerate(bnames)
            }
            ident_t = rp_t[:, _IDENT_OFF : _IDENT_OFF + 64].bitcast(bf16)

            # ---- projections (feat-major, TF32 matmuls, bf16 outputs) ----
            def project(wname, bname, func, outname, do_scale):
                seq = seqp.tile([P, L], bf16, name=outname)
                for th in range(L // TH):
                    ps = big_ps.tile([P, TH], f32, tag="big", name=f"{outname}_ps{th}")
                    for e in range(4):
                        nc.tensor.matmul(
                            ps[:],
                            wt[wname][e],
                            xts[e][:, th * TH : (th + 1) * TH],
                            start=(e == 0),
                            stop=(e == 3),
                        )
                    sl = seq[:, th * TH : (th + 1) * TH]
                    nc.scalar.activation(sl, ps[:], func, bias=bt[bname])
                    if do_scale:
                        nc.gpsimd.tensor_mul(sl, sl, scb_t[:, th * TH : (th + 1) * TH])
                return seq

            q_seq = {}
            k_seq = {}
            for h in "ab":
                q_seq[h] = project(f"wq_{h}", f"bq_{h}", AF.Relu, f"q_{h}", True)
                k_seq[h] = project(f"wk_{h}", f"bk_{h}", AF.Relu, f"k_{h}", True)
            v_seq = project("wv", "bv", AF.Identity, "v_pair", False)

            # ---- attention ----
            # Per-head fp32 running state, each in its own persistent PSUM
            # bank (start=True zeroes a whole 2KB bank region, so groups can
            # never share a bank); bf16 snapshots feed the next chunk's inter.
            s_bank = {
                "a": s_ps.tile([P, D + 1], f32, name="s_bank_a"),
                "b": s_ps.tile([P, D + 1], f32, name="s_bank_b"),
            }
            S_prev = {"a": None, "b": None}
            for c in range(NCHUNK):
                cs = slice(c * P, (c + 1) * P)
                vt_ps = sq_ps.tile([P, P], bf16, tag="sq", name=f"vtps{c}")
                nc.tensor.matmul(vt_ps[:], v_seq[:, cs], ident_t, is_transpose=True)
                vtile = {}
                for j, h in enumerate("ab"):
                    vt = vtp.tile([P, D + 1], bf16, tag="vt", name=f"vt_{h}{c}")
                    nc.vector.tensor_copy(vt[:, 0:D], vt_ps[:, j * D : (j + 1) * D])
                    nc.gpsimd.memset(vt[:, D : D + 1], 1.0)
                    vtile[h] = vt

                attn_pair = app.tile([P, P], bf16, tag="ap", name=f"ap{c}")
                for j, h in enumerate("ab"):
                    qc = q_seq[h][:, cs]
                    kc = k_seq[h][:, cs]
                    # masked A^T
                    b_ps = sq_ps.tile([P, P], f32, tag="sq", name=f"bps_{h}{c}")
                    nc.tensor.matmul(b_ps[:], kc, qc, start=True, stop=True)
                    b_sb = bp.tile([P, P], bf16, tag="bsb", name=f"bsb_{h}{c}")
                    nc.vector.tensor_mul(b_sb[:], b_ps[:], mask_t)
                    # qkv = intra + inter
                    qkv = acc_ps.tile([P, D + 1], f32, tag="acc", name=f"qkv_{h}{c}")
                    nc.tensor.matmul(
                        qkv[:], b_sb[:], vtile[h][:], start=True, stop=(c == 0)
                    )
                    if c > 0:
                        nc.tensor.matmul(
                            qkv[:], qc, S_prev[h][:], start=False, stop=True
                        )
                    # state update in persistent PSUM (skip on last chunk)
                    if c < NCHUNK - 1:
                        kt_ps = sq_ps.tile([P, P], bf16, tag="sq", name=f"ktps_{h}{c}")
                        nc.tensor.matmul(kt_ps[:], kc, ident_t, is_transpose=True)
                        ktok = ktokp.tile([P, P], bf16, tag="ktok", name=f"ktok_{h}{c}")
                        nc.vector.tensor_copy(ktok[:], kt_ps[:])
                        s_slice = s_bank[h][:]
                        nc.tensor.matmul(
                            s_slice,
                            ktok[:],
                            vtile[h][:],
                            start=(c == 0),
                            stop=(c == NCHUNK - 2),
                            skip_group_check=True,
                        )
                        s_new = sp.tile([P, D + 1], bf16, tag="S", name=f"S_{h}{c}")
                        nc.scalar.copy(s_new[:], s_slice)
                        S_prev[h] = s_new
                    # normalize
                    r_col = rp.tile([P, 2], f32, tag="r", name=f"r_{h}{c}")
                    nc.vector.tensor_scalar_max(r_col[:, 0:1], qkv[:, D : D + 1], EPS)
                    nc.vector.reciprocal(r_col[:, 1:2], r_col[:, 0:1])
                    nc.vector.tensor_scalar_mul(
                        attn_pair[:, j * D : (j + 1) * D], qkv[:, 0:D], r_col[:, 1:2]
                    )
                # out projection for this chunk (TF32)
                at_ps = sq_ps.tile([P, P], bf16, tag="sq", name=f"atps{c}")
                nc.tensor.matmul(at_ps[:], attn_pair[:], ident_t, is_transpose=True)
                at_sb = atp.tile([P, P], f32r, tag="at", name=f"at{c}")
                nc.scalar.copy(at_sb[:], at_ps[:])
                o_ps = big_ps.tile([P, E], f32, tag="big", name=f"ops{c}")
                nc.tensor.matmul(o_ps[:], at_sb[:], wo_t, start=True, stop=True)
                o_sb = outp.tile([P, E], f32, tag="osb", name=f"osb{c}")
                nc.scalar.copy(o_sb[:], o_ps[:])
                nc.gpsimd.dma_start(out_d[cs, :], o_sb[:])

    nc.compile()
    return nc


def _get_nc():
    if "nc" not in _CACHE:
        _CACHE["nc"] = _build_bass()
    return _CACHE["nc"]


def make_in_maps(query, Wq, bq, Wk, bk, Wv, bv, Wo, bo):
    import ml_dtypes

    f32 = np.float32
    query = np.asarray(query, f32)
    x3 = query.reshape(L, BSZ, E)  # faithful torch .view reshape
    idx = (np.pi / 2) * np.arange(1, L + 1, dtype=f32) / f32(L)
    sinv = np.sin(idx).astype(f32)
    cosv = np.cos(idx).astype(f32)

    Wq, Wk, Wv, Wo = (np.asarray(w, f32) for w in (Wq, Wk, Wv, Wo))
    bq, bk, bv = (np.asarray(b, f32) for b in (bq, bk, bv))

    def wslice_dup(W, h):
        """(128, 512): [Wh.T | Wh.T] dup cols laid out as 4 e-tiles of 128."""
        w = W[D * h : D * (h + 1), :].T  # (512, 64)
        wd = np.concatenate([w, w], axis=1)  # (512, 128)
        return np.hstack([wd[e * P : (e + 1) * P, :] for e in range(4)])

    def bdup(b, h):
        bb = b[D * h : D * (h + 1)]
        return np.concatenate([bb, bb]).astype(f32)

    ident_f32 = np.ascontiguousarray(np.eye(P, dtype=ml_dtypes.bfloat16)).view(f32)

    in_maps = []
    for c in range(N_CORES):
        i, p = divmod(c, 4)
        hA, hB = 2 * p, 2 * p + 1

        cw = np.hstack(
            [wslice_dup(Wq, hA), wslice_dup(Wq, hB), wslice_dup(Wk, hA), wslice_dup(Wk, hB)]
        )  # (128, 2048)
        assert cw.shape == (P, 2048), cw.shape

        wv_p = Wv[P * p : P * (p + 1), :].T  # (512, 128)
        wv_pack = np.hstack([wv_p[e * P : (e + 1) * P, :] for e in range(4)])  # (128,512)
        wo_pack = Wo[:, P * p : P * (p + 1)].T  # (128, 512)
        scb = np.empty((P, L), f32)
        scb[0:D] = sinv[None, :]
        scb[D:P] = cosv[None, :]
        mask = np.triu(np.ones((P, P), f32))
        biases = np.stack(
            [bdup(bq, hA), bdup(bq, hB), bdup(bk, hA), bdup(bk, hB), bv[P * p : P * (p + 1)]],
            axis=1,
        ).astype(f32)  # (128, 5)
        rest = np.hstack([wv_pack, wo_pack, scb, mask, biases, ident_f32])
        assert rest.shape == (P, _REST_COLS), rest.shape

        in_maps.append(
            dict(
                xt=np.ascontiguousarray(x3[:, i, :].T),
                cw=np.ascontiguousarray(cw),
                rp=np.ascontiguousarray(rest),
            )
        )
    return in_maps


def assemble(partials, bo):
    out_flat = np.zeros((BSZ * L, E), np.float32)
    out_flat[0::2] = partials[0] + partials[1] + partials[2] + partials[3]
    out_flat[1::2] = partials[4] + partials[5] + partials[6] + partials[7]
    out_flat += np.asarray(bo, np.float32)[None, :]
    return out_flat.reshape(BSZ, L, E)


def run(inputs, trace=False):
    from concourse.bass_utils import run_bass_kernel_spmd

    in_maps = make_in_maps(**inputs)
    nc = _get_nc()
    res = run_bass_kernel_spmd(nc, in_maps, list(range(N_CORES)), trace=trace)
    partials = [r["out"] for r in res.results]
    return assemble(partials, inputs["bo"]), res


def kernel(**inputs):
    out, _ = run(inputs, trace=False)
    return out
